# revision 33
# baseline (speedup 1.0000x reference)
"""Trainium2 Bass kernel for masked ALiBi attention (returns out AND p_attn).

Problem: B=2, H=16, S=2048, D=64.
  scores = QK^T/8 + alibi*slope_h, masked (mask==0 -> -1e9), p = softmax,
  out = p @ V.  Returns (out [B,H,S,D] f32, p [B,H,S,S] f32).

Sharding (8 cores, no collectives): core c -> b = c>>2, head-half hh=(c>>1)&1,
query-half qh=c&1.  Each core: 8 heads x 1024 q-rows x full 2048 k.
This minimizes HBM traffic (92 MiB/core) while amortizing per-head transposes
and per-qtile mask/alibi preprocessing.

Per-core algorithm (per head h, per 128-row q-tile, split in 2 halves of
1024 k for PSUM double-buffering; q-tiles processed in groups of GW for a
shared P@V with wider matmuls):
  KT/QT: fp16 via PE transposes (Q pre-scaled by 0.125/slope_h on copy-out);
    next head's loads+transposes are emitted ahead (software pipelining)
  combined = alibi + 30000*mask       (DVE f32, once per q-tile, all heads)
  scores = QT^T@KT (fp16 matmul, fp32 PSUM); scores += combined (DVE, in PSUM)
  p_f16 = exp(slope*scores - 30000*slope)   (ACT; mask -> exp underflow -> 0)
  PT = p^T via PE transpose-mode (fp16 128x128 chunks -> PSUM -> SBUF)
  outT[65,128*GW] = sum_k [V|1]^T @ PT  -> row 64 = softmax denominators
  transpose outT cols -> [128,65]; recip = 1/col64 (exact denominator);
  p_out = p_f16*recip -> DMA; out = outT rows 0..63 * recip -> DMA
"""

import os
import sys

sys.path.insert(0, "/opt/trn_rl_repo")

import numpy as np

import concourse.bass as bass
import concourse.mybir as mybir
import concourse.tile as tile
from concourse import bacc
from concourse.bass_utils import run_bass_kernel_spmd
from concourse.masks import make_identity

B, H, S, D = 2, 16, 2048, 64
NCORES = 8
HLOC = 8          # heads per core
QLOC = 1024       # q rows per core
NQT = QLOC // 128  # 8 q-tiles per core
NKC = S // 128     # 16 k-chunks
MASK_BIG = 30000.0

f32 = mybir.dt.float32
f16 = mybir.dt.float16
i32 = mybir.dt.int32
u32 = mybir.dt.uint32
EXP = mybir.ActivationFunctionType.Exp
MULT = mybir.AluOpType.mult
ADD = mybir.AluOpType.add

GW = int(os.environ.get("BASS_GW", "2"))      # q-tiles per PV group
NORM_ON_GPSIMD = os.environ.get("BASS_NORM_GPS", "0") == "1"


def _slopes():
    start = 2.0 ** (-8.0 / H)
    return start ** np.arange(1, H + 1, dtype=np.float64)


def build_kernel(tc, aps, hloc=HLOC, nqt=NQT):
    nc = tc.nc
    q_d, k_d, v_d, m_d, a_d, qs_d, es_d, eb_d, p_d, o_d = aps
    from contextlib import ExitStack

    ctx = ExitStack()
    with ctx:
        const = ctx.enter_context(tc.tile_pool(name="const", bufs=1))
        comb_pool = ctx.enter_context(tc.tile_pool(name="comb", bufs=1))
        kt_pool = ctx.enter_context(tc.tile_pool(name="kt", bufs=2))
        qt_pool = ctx.enter_context(tc.tile_pool(name="qt", bufs=2))
        ld_pool = ctx.enter_context(tc.tile_pool(name="ld", bufs=2))
        qld_pool = ctx.enter_context(tc.tile_pool(name="qld", bufs=3))
        work = ctx.enter_context(tc.tile_pool(name="work", bufs=2))
        pbf_pool = ctx.enter_context(tc.tile_pool(name="pbf", bufs=9))
        pout_pool = ctx.enter_context(tc.tile_pool(name="pout", bufs=2))
        small = ctx.enter_context(tc.tile_pool(name="small", bufs=6))
        psum = ctx.enter_context(tc.tile_pool(name="psum", bufs=2, space="PSUM"))
        psumx = ctx.enter_context(tc.tile_pool(name="psumx", bufs=3, space="PSUM"))
        psumst = ctx.enter_context(tc.tile_pool(name="psumst", bufs=1, space="PSUM"))

        # constants
        ident_f32 = const.tile([128, 128], f32)
        make_identity(nc, ident_f32[:])
        ident_f16 = const.tile([128, 128], f16)
        nc.vector.tensor_copy(ident_f16[:], ident_f32[:])
        qs_sb = const.tile([64, hloc], f32)
        nc.sync.dma_start(qs_sb[:], qs_d[:])
        es_sb = const.tile([128, hloc], f32)
        nc.sync.dma_start(es_sb[:], es_d[:])
        eb_sb = const.tile([128, hloc], f32)
        nc.sync.dma_start(eb_sb[:], eb_d[:])

        def head_setup(h):
            # ---- per-head setup: load K/V f32 (fast HWDGE), cast to fp16,
            # build KT and scaled QT
            k_raw = ld_pool.tile([128, NKC, 64], f32, tag="raw")
            nc.sync.dma_start(
                out=k_raw[:], in_=k_d[h].rearrange("(c p) d -> p c d", p=128)
            )
            k_st = ld_pool.tile([128, NKC, 64], f16, tag="kst")
            nc.any.tensor_copy(k_st[:], k_raw[:])
            # V with an appended ones-column: PV then also yields softmax sums
            v_raw = ld_pool.tile([128, NKC, 64], f32, tag="raw")
            nc.sync.dma_start(
                out=v_raw[:], in_=v_d[h].rearrange("(c p) d -> p c d", p=128)
            )
            v_f16 = ld_pool.tile([128, NKC, 65], f16, tag="vf")
            nc.any.tensor_copy(v_f16[:, :, 0:64], v_raw[:])
            nc.vector.memset(v_f16[:, :, 64:65], 1.0)
            # kt/qt replicated into both partition halves so consecutive QK
            # matmuls target alternating row-groups (LDWEIGHTS pipelining)
            kt = kt_pool.tile([64, S], f16, tag="kt")
            for r in range(4):
                st = psumst.tile([64, 512], f16, tag="stage")
                for j in range(4):
                    c = r * 4 + j
                    nc.tensor.transpose(
                        st[:, j * 128:(j + 1) * 128],
                        k_st[:, c, :],
                        ident_f16[:],
                    )
                nc.any.tensor_copy(kt[:, r * 512:(r + 1) * 512], st[:])
            qt_t = qt_pool.tile([64, nqt * 128], f16, tag="qt")
            for r in range((nqt + 3) // 4):
                nj = min(4, nqt - r * 4)
                st = psumst.tile([64, 512], f16, tag="stage")
                for j in range(nj):
                    qi = r * 4 + j
                    q_raw = qld_pool.tile([128, 64], f32, tag="qraw")
                    nc.sync.dma_start(
                        out=q_raw[:], in_=q_d[h, qi * 128:(qi + 1) * 128, :]
                    )
                    q_st = qld_pool.tile([128, 64], f16, tag="qst")
                    nc.any.tensor_copy(q_st[:], q_raw[:])
                    nc.tensor.transpose(
                        st[:, j * 128:(j + 1) * 128], q_st[:], ident_f16[:]
                    )
                nc.vector.tensor_scalar(
                    out=qt_t[:, r * 512:r * 512 + nj * 128],
                    in0=st[:, : nj * 128],
                    scalar1=qs_sb[:, h:h + 1],
                    scalar2=None,
                    op0=MULT,
                )
            return kt, qt_t, v_f16

        # combined[qt] = alibi + 30000*mask  (f32, resident, shared by all
        # heads).  Build the first group's q-tiles before head-0's K/V loads
        # hit the DMA queues so the first adds aren't starved at startup.
        comb = comb_pool.tile([128, nqt * 2048], f32)

        def build_comb(qt):
            cs = comb[:, qt * 2048:(qt + 1) * 2048]
            nc.sync.dma_start(cs, a_d[qt * 128:(qt + 1) * 128, :])
            m_st = ld_pool.tile([128, 2048], i32, tag="mst")
            nc.sync.dma_start(m_st[:], m_d[qt * 128:(qt + 1) * 128, :])
            nc.vector.scalar_tensor_tensor(
                out=cs, in0=m_st[:], scalar=MASK_BIG, in1=cs, op0=MULT, op1=ADD
            )

        for qt in range(GW):
            build_comb(qt)
        setup = head_setup(0)
        for qt in range(GW, nqt):
            build_comb(qt)
        for h in range(hloc):
            kt, qt_t, v_f16 = setup
            if h + 1 < hloc:
                setup = head_setup(h + 1)

            for g in range(nqt // GW):
                # ---- group of GW q-tiles: shared PV with N=128*GW matmuls
                # pt4 layout: chunk c cols [c*128*GW + t*128 : .. +128] = tile t
                pt4 = work.tile([128, NKC * 128 * GW], f16, tag="ptsb")
                p_sbs = []
                for t in range(GW):
                    qi = g * GW + t
                    lhsT = qt_t[:, qi * 128:(qi + 1) * 128]
                    p_sb = pbf_pool.tile([128, S], f16, tag="pbf")
                    p_sbs.append(p_sb)
                    for half in range(2):
                        sc = psum.tile([128, 1024], f32, tag="scores")
                        for j in range(2):
                            kcol = half * 1024 + j * 512
                            nc.tensor.matmul(
                                sc[:, j * 512:(j + 1) * 512],
                                lhsT=lhsT,
                                rhs=kt[:, kcol:kcol + 512],
                                start=True,
                                stop=True,
                            )
                        cslice = comb[
                            :,
                            qi * 2048 + half * 1024:qi * 2048 + half * 1024 + 1024,
                        ]
                        nc.vector.tensor_add(sc[:], sc[:], cslice)
                        nc.scalar.activation(
                            p_sb[:, half * 1024:(half + 1) * 1024],
                            sc[:],
                            EXP,
                            bias=eb_sb[:, h:h + 1],
                            scale=es_sb[:, h:h + 1],
                        )
                        # transpose p (fp16) for the PV matmul
                        pt_ps = psumx.tile([128, 1024], f16, tag="ptx")
                        for c in range(8):
                            col = half * 1024 + c * 128
                            nc.tensor.transpose(
                                pt_ps[:, c * 128:(c + 1) * 128],
                                p_sb[:, col:col + 128],
                                ident_f16[:],
                            )
                        # scatter 8 chunks into pt4 (chunk-major, tile t slot)
                        # bitcast fp16->uint32 halves the element count
                        dst = pt4[:, :].rearrange(
                            "p (c w) -> p c w", w=128 * GW
                        )[:, half * 8:half * 8 + 8, t * 128:(t + 1) * 128]
                        nc.any.tensor_copy(
                            dst,
                            pt_ps[:].rearrange("p (c w) -> p c w", w=128),
                        )
                # PV: 16 chunk matmuls of N=128*GW, accumulating
                outT = psumx.tile([65, 128 * GW], f32, tag="ptx")
                for cg in range(NKC):
                    nc.tensor.matmul(
                        outT[:],
                        lhsT=v_f16[:, cg, :],
                        rhs=pt4[:, cg * 128 * GW:(cg + 1) * 128 * GW],
                        start=(cg == 0),
                        stop=(cg == NKC - 1),
                    )
                # outT[64,:] = softmax denominators; transpose per tile, recip,
                # normalize p and out
                outT_sb = small.tile([65, 128 * GW], f32, tag="otsb")
                nc.any.tensor_copy(outT_sb[:], outT[:])
                for t in range(GW):
                    qi = g * GW + t
                    fix_ps = psumx.tile([128, 65], f32, tag="ptx")
                    nc.tensor.transpose(
                        fix_ps[:],
                        outT_sb[:, t * 128:(t + 1) * 128],
                        ident_f32[:65, :65],
                    )
                    recip = small.tile([128, 1], f32, tag="rs")
                    nc.vector.reciprocal(recip[:], fix_ps[:, 64:65])
                    p_out = pout_pool.tile([128, S], f32, tag="pout")
                    nc.any.tensor_scalar(
                        out=p_out[:],
                        in0=p_sbs[t][:],
                        scalar1=recip[:],
                        scalar2=None,
                        op0=MULT,
                    )
                    out_sb = small.tile([128, 64], f32, tag="osb")
                    nc.any.tensor_scalar(
                        out=out_sb[:],
                        in0=fix_ps[:, 0:64],
                        scalar1=recip[:],
                        scalar2=None,
                        op0=MULT,
                    )
                    nc.sync.dma_start(
                        p_d[h, qi * 128:(qi + 1) * 128, :], p_out[:]
                    )
                    nc.sync.dma_start(
                        o_d[h, qi * 128:(qi + 1) * 128, :], out_sb[:]
                    )


def build_program(hloc=HLOC, qloc=QLOC):
    nqt = qloc // 128
    nc = bacc.Bacc(
        "TRN2", target_bir_lowering=False, debug=False, num_devices=NCORES
    )
    q_d = nc.dram_tensor("q", [hloc, qloc, D], f32, kind="ExternalInput").ap()
    k_d = nc.dram_tensor("k", [hloc, S, D], f32, kind="ExternalInput").ap()
    v_d = nc.dram_tensor("v", [hloc, S, D], f32, kind="ExternalInput").ap()
    m_d = nc.dram_tensor("mask", [qloc, S], i32, kind="ExternalInput").ap()
    a_d = nc.dram_tensor("alibi", [qloc, S], f32, kind="ExternalInput").ap()
    qs_d = nc.dram_tensor("qscale", [64, hloc], f32, kind="ExternalInput").ap()
    es_d = nc.dram_tensor("escale", [128, hloc], f32, kind="ExternalInput").ap()
    eb_d = nc.dram_tensor("ebias", [128, hloc], f32, kind="ExternalInput").ap()
    p_d = nc.dram_tensor("p", [hloc, qloc, S], f32, kind="ExternalOutput").ap()
    o_d = nc.dram_tensor("o", [hloc, qloc, D], f32, kind="ExternalOutput").ap()
    aps = (q_d, k_d, v_d, m_d, a_d, qs_d, es_d, eb_d, p_d, o_d)
    with tile.TileContext(nc) as tc:
        build_kernel(tc, aps, hloc=hloc, nqt=nqt)
    nc.compile()
    return nc


_CACHE = {}


def _get_program():
    if "nc" not in _CACHE:
        _CACHE["nc"] = build_program()
    return _CACHE["nc"]


def _make_in_maps(query, key, value, mask, alibi):
    slopes = _slopes()
    in_maps = []
    for c in range(NCORES):
        b, hh, qh = c >> 2, (c >> 1) & 1, c & 1
        h0, q0 = hh * HLOC, qh * QLOC
        sl = slopes[h0:h0 + HLOC]
        in_maps.append({
            "q": np.ascontiguousarray(
                query[b, h0:h0 + HLOC, q0:q0 + QLOC, :], dtype=np.float32
            ),
            "k": np.ascontiguousarray(key[b, h0:h0 + HLOC], dtype=np.float32),
            "v": np.ascontiguousarray(value[b, h0:h0 + HLOC], dtype=np.float32),
            "mask": np.ascontiguousarray(
                mask[b, 0, q0:q0 + QLOC, :], dtype=np.int32
            ),
            "alibi": np.ascontiguousarray(
                alibi[b, q0:q0 + QLOC, :], dtype=np.float32
            ),
            "qscale": np.tile(
                (0.125 / sl).astype(np.float32)[None, :], (64, 1)
            ),
            "escale": np.tile(sl.astype(np.float32)[None, :], (128, 1)),
            "ebias": np.tile(
                (-MASK_BIG * sl).astype(np.float32)[None, :], (128, 1)
            ),
        })
    return in_maps


def _gather(results):
    out = np.zeros((B, H, S, D), np.float32)
    p = np.zeros((B, H, S, S), np.float32)
    for c in range(NCORES):
        b, hh, qh = c >> 2, (c >> 1) & 1, c & 1
        h0, q0 = hh * HLOC, qh * QLOC
        out[b, h0:h0 + HLOC, q0:q0 + QLOC, :] = results[c]["o"]
        p[b, h0:h0 + HLOC, q0:q0 + QLOC, :] = results[c]["p"]
    return out, p


def kernel(query, key, value, mask, alibi):
    nc = _get_program()
    in_maps = _make_in_maps(query, key, value, mask, alibi)
    res = run_bass_kernel_spmd(nc, in_maps, core_ids=list(range(NCORES)))
    return _gather(res.results)


def _ensure_ntff_hook():
    """Wire the axon NTFF profile hook into the stub antenv package."""
    import types

    try:
        import antenv.axon_hooks  # noqa: F401

        return
    except ImportError:
        pass
    import antenv
    from trn_agent_boot.trn_boot import _ntff_profile_via_ctypes

    hook = _ntff_profile_via_ctypes("/opt/axon/libaxon_pjrt.so")
    mod = types.ModuleType("antenv.axon_hooks")
    mod.get_axon_ntff_profile_hook = lambda: hook
    mod.set_axon_ntff_profile_hook = lambda h: None
    sys.modules["antenv.axon_hooks"] = mod
    antenv.axon_hooks = mod

    import concourse.bass_utils as bu

    if not getattr(bu, "_upload_patched", False):
        orig = bu.upload_artifacts

        def _safe_upload(tmpdir):
            try:
                return orig(tmpdir)
            except Exception as e:  # no artifact bucket in this container
                return f"upload-skipped: {e}"

        bu.upload_artifacts = _safe_upload
        bu._upload_patched = True


def kernel_traced(query, key, value, mask, alibi, **kw):
    """Like kernel(), but with NTFF profiling; returns (outputs, BassKernelResults)."""
    _ensure_ntff_hook()
    nc = _get_program()
    in_maps = _make_in_maps(query, key, value, mask, alibi)
    res = run_bass_kernel_spmd(
        nc, in_maps, core_ids=list(range(NCORES)), trace=True, **kw
    )
    return _gather(res.results), res


# revision 34
# speedup vs baseline: 1.0379x; 1.0379x over previous
"""Trainium2 Bass kernel for masked ALiBi attention (returns out AND p_attn).

Problem: B=2, H=16, S=2048, D=64.
  scores = QK^T/8 + alibi*slope_h, masked (mask==0 -> -1e9), p = softmax,
  out = p @ V.  Returns (out [B,H,S,D] f32, p [B,H,S,S] f32).

Sharding (8 cores, no collectives): core c -> b = c>>2, head-half hh=(c>>1)&1,
query-half qh=c&1.  Each core: 8 heads x 1024 q-rows x full 2048 k.
This minimizes HBM traffic (92 MiB/core) while amortizing per-head transposes
and per-qtile mask/alibi preprocessing.

Per-core algorithm (per head h, per 128-row q-tile, split in 2 halves of
1024 k for PSUM double-buffering; q-tiles processed in groups of GW for a
shared P@V with wider matmuls):
  KT/QT: fp16 via PE transposes (Q pre-scaled by 0.125/slope_h on copy-out);
    next head's loads+transposes are emitted ahead (software pipelining)
  combined = alibi + 30000*mask       (DVE f32, once per q-tile, all heads)
  scores = QT^T@KT (fp16 matmul, fp32 PSUM); scores += combined (DVE, in PSUM)
  p_f16 = exp(slope*scores - 30000*slope)   (ACT; mask -> exp underflow -> 0)
  PT = p^T via PE transpose-mode (fp16 128x128 chunks -> PSUM -> SBUF)
  outT[65,128*GW] = sum_k [V|1]^T @ PT  -> row 64 = softmax denominators
  transpose outT cols -> [128,65]; recip = 1/col64 (exact denominator);
  p_out = p_f16*recip -> DMA; out = outT rows 0..63 * recip -> DMA
"""

import os
import sys

sys.path.insert(0, "/opt/trn_rl_repo")

import numpy as np

import concourse.bass as bass
import concourse.mybir as mybir
import concourse.tile as tile
from concourse import bacc
from concourse.bass_utils import run_bass_kernel_spmd
from concourse.masks import make_identity

B, H, S, D = 2, 16, 2048, 64
NCORES = 8
HLOC = 8          # heads per core
QLOC = 1024       # q rows per core
NQT = QLOC // 128  # 8 q-tiles per core
NKC = S // 128     # 16 k-chunks
MASK_BIG = 30000.0

f32 = mybir.dt.float32
f16 = mybir.dt.float16
i32 = mybir.dt.int32
u32 = mybir.dt.uint32
EXP = mybir.ActivationFunctionType.Exp
MULT = mybir.AluOpType.mult
ADD = mybir.AluOpType.add

GW = int(os.environ.get("BASS_GW", "2"))      # q-tiles per PV group
NORM_ON_GPSIMD = os.environ.get("BASS_NORM_GPS", "0") == "1"


def _slopes():
    start = 2.0 ** (-8.0 / H)
    return start ** np.arange(1, H + 1, dtype=np.float64)


def build_kernel(tc, aps, hloc=HLOC, nqt=NQT):
    nc = tc.nc
    q_d, k_d, v_d, m_d, a_d, qs_d, es_d, eb_d, p_d, o_d = aps
    from contextlib import ExitStack

    ctx = ExitStack()
    with ctx:
        const = ctx.enter_context(tc.tile_pool(name="const", bufs=1))
        comb_pool = ctx.enter_context(tc.tile_pool(name="comb", bufs=1))
        kt_pool = ctx.enter_context(tc.tile_pool(name="kt", bufs=2))
        qt_pool = ctx.enter_context(tc.tile_pool(name="qt", bufs=2))
        ld_pool = ctx.enter_context(tc.tile_pool(name="ld", bufs=2))
        qld_pool = ctx.enter_context(tc.tile_pool(name="qld", bufs=3))
        work = ctx.enter_context(tc.tile_pool(name="work", bufs=2))
        pbf_pool = ctx.enter_context(tc.tile_pool(name="pbf", bufs=8))
        pout_pool = ctx.enter_context(tc.tile_pool(name="pout", bufs=2))
        small = ctx.enter_context(tc.tile_pool(name="small", bufs=6))
        psum = ctx.enter_context(tc.tile_pool(name="psum", bufs=2, space="PSUM"))
        psumx = ctx.enter_context(tc.tile_pool(name="psumx", bufs=3, space="PSUM"))
        psumst = ctx.enter_context(tc.tile_pool(name="psumst", bufs=1, space="PSUM"))

        # constants
        ident_f32 = const.tile([128, 128], f32)
        make_identity(nc, ident_f32[:])
        ident_f16 = const.tile([128, 128], f16)
        nc.vector.tensor_copy(ident_f16[:], ident_f32[:])
        qs_sb = const.tile([64, hloc], f32)
        nc.sync.dma_start(qs_sb[:], qs_d[:])
        es_sb = const.tile([128, hloc], f32)
        nc.sync.dma_start(es_sb[:], es_d[:])
        eb_sb = const.tile([128, hloc], f32)
        nc.sync.dma_start(eb_sb[:], eb_d[:])

        def head_setup(h):
            # ---- per-head setup: load K/V f32 (fast HWDGE), cast to fp16,
            # build KT and scaled QT
            k_raw = ld_pool.tile([128, NKC, 64], f32, tag="raw")
            nc.sync.dma_start(
                out=k_raw[:], in_=k_d[h].rearrange("(c p) d -> p c d", p=128)
            )
            k_st = ld_pool.tile([128, NKC, 64], f16, tag="kst")
            nc.any.tensor_copy(k_st[:], k_raw[:])
            # V with an appended ones-column: PV then also yields softmax sums
            v_raw = ld_pool.tile([128, NKC, 64], f32, tag="raw")
            nc.sync.dma_start(
                out=v_raw[:], in_=v_d[h].rearrange("(c p) d -> p c d", p=128)
            )
            v_f16 = ld_pool.tile([128, NKC, 65], f16, tag="vf")
            nc.any.tensor_copy(v_f16[:, :, 0:64], v_raw[:])
            nc.vector.memset(v_f16[:, :, 64:65], 1.0)
            # kt/qt replicated into both partition halves so consecutive QK
            # matmuls target alternating row-groups (LDWEIGHTS pipelining)
            kt = kt_pool.tile([64, S], f16, tag="kt")
            for r in range(4):
                st = psumst.tile([64, 512], f16, tag="stage")
                for j in range(4):
                    c = r * 4 + j
                    nc.tensor.transpose(
                        st[:, j * 128:(j + 1) * 128],
                        k_st[:, c, :],
                        ident_f16[:],
                    )
                nc.any.tensor_copy(kt[:, r * 512:(r + 1) * 512], st[:])
            qt_t = qt_pool.tile([64, nqt * 128], f16, tag="qt")
            for r in range((nqt + 3) // 4):
                nj = min(4, nqt - r * 4)
                st = psumst.tile([64, 512], f16, tag="stage")
                for j in range(nj):
                    qi = r * 4 + j
                    q_raw = qld_pool.tile([128, 64], f32, tag="qraw")
                    nc.sync.dma_start(
                        out=q_raw[:], in_=q_d[h, qi * 128:(qi + 1) * 128, :]
                    )
                    q_st = qld_pool.tile([128, 64], f16, tag="qst")
                    nc.any.tensor_copy(q_st[:], q_raw[:])
                    nc.tensor.transpose(
                        st[:, j * 128:(j + 1) * 128], q_st[:], ident_f16[:]
                    )
                nc.vector.tensor_scalar(
                    out=qt_t[:, r * 512:r * 512 + nj * 128],
                    in0=st[:, : nj * 128],
                    scalar1=qs_sb[:, h:h + 1],
                    scalar2=None,
                    op0=MULT,
                )
            return kt, qt_t, v_f16

        setup = head_setup(0)
        # combined[qt] = alibi + 30000*mask  (f32, resident, shared by all heads)
        comb = comb_pool.tile([128, nqt * 2048], f32)
        for qt in range(nqt):
            cs = comb[:, qt * 2048:(qt + 1) * 2048]
            nc.sync.dma_start(cs, a_d[qt * 128:(qt + 1) * 128, :])
            m_st = ld_pool.tile([128, 2048], i32, tag="mst")
            nc.sync.dma_start(m_st[:], m_d[qt * 128:(qt + 1) * 128, :])
            nc.vector.scalar_tensor_tensor(
                out=cs, in0=m_st[:], scalar=MASK_BIG, in1=cs, op0=MULT, op1=ADD
            )
        for h in range(hloc):
            kt, qt_t, v_f16 = setup
            if h + 1 < hloc:
                setup = head_setup(h + 1)

            for g in range(nqt // GW):
                # ---- group of GW q-tiles: shared PV with N=128*GW matmuls
                # pt4 layout: chunk c cols [c*128*GW + t*128 : .. +128] = tile t
                pt4 = work.tile([128, NKC * 128 * GW], f16, tag="ptsb")
                p_sbs = []
                for t in range(GW):
                    qi = g * GW + t
                    lhsT = qt_t[:, qi * 128:(qi + 1) * 128]
                    p_sb = pbf_pool.tile([128, S], f16, tag="pbf")
                    p_sbs.append(p_sb)
                    for half in range(2):
                        sc = psum.tile([128, 1024], f32, tag="scores")
                        for j in range(2):
                            kcol = half * 1024 + j * 512
                            nc.tensor.matmul(
                                sc[:, j * 512:(j + 1) * 512],
                                lhsT=lhsT,
                                rhs=kt[:, kcol:kcol + 512],
                                start=True,
                                stop=True,
                            )
                        cslice = comb[
                            :,
                            qi * 2048 + half * 1024:qi * 2048 + half * 1024 + 1024,
                        ]
                        nc.vector.tensor_add(sc[:], sc[:], cslice)
                        nc.scalar.activation(
                            p_sb[:, half * 1024:(half + 1) * 1024],
                            sc[:],
                            EXP,
                            bias=eb_sb[:, h:h + 1],
                            scale=es_sb[:, h:h + 1],
                        )
                        # transpose p (fp16) for the PV matmul
                        pt_ps = psumx.tile([128, 1024], f16, tag="ptx")
                        for c in range(8):
                            col = half * 1024 + c * 128
                            nc.tensor.transpose(
                                pt_ps[:, c * 128:(c + 1) * 128],
                                p_sb[:, col:col + 128],
                                ident_f16[:],
                            )
                        # scatter 8 chunks into pt4 (chunk-major, tile t slot)
                        # bitcast fp16->uint32 halves the element count
                        dst = pt4[:, :].rearrange(
                            "p (c w) -> p c w", w=128 * GW
                        )[:, half * 8:half * 8 + 8, t * 128:(t + 1) * 128]
                        nc.any.tensor_copy(
                            dst,
                            pt_ps[:].rearrange("p (c w) -> p c w", w=128),
                        )
                # PV: 16 chunk matmuls of N=128*GW, accumulating
                outT = psumx.tile([65, 128 * GW], f32, tag="ptx")
                for cg in range(NKC):
                    nc.tensor.matmul(
                        outT[:],
                        lhsT=v_f16[:, cg, :],
                        rhs=pt4[:, cg * 128 * GW:(cg + 1) * 128 * GW],
                        start=(cg == 0),
                        stop=(cg == NKC - 1),
                    )
                # outT[64,:] = softmax denominators; transpose per tile, recip,
                # normalize p and out
                outT_sb = small.tile([65, 128 * GW], f32, tag="otsb")
                nc.any.tensor_copy(outT_sb[:], outT[:])
                for t in range(GW):
                    qi = g * GW + t
                    fix_ps = psumx.tile([128, 65], f32, tag="ptx")
                    nc.tensor.transpose(
                        fix_ps[:],
                        outT_sb[:, t * 128:(t + 1) * 128],
                        ident_f32[:65, :65],
                    )
                    recip = small.tile([128, 1], f32, tag="rs")
                    nc.vector.reciprocal(recip[:], fix_ps[:, 64:65])
                    p_out = pout_pool.tile([128, S], f32, tag="pout")
                    nc.any.tensor_scalar(
                        out=p_out[:],
                        in0=p_sbs[t][:],
                        scalar1=recip[:],
                        scalar2=None,
                        op0=MULT,
                    )
                    out_sb = small.tile([128, 64], f32, tag="osb")
                    nc.any.tensor_scalar(
                        out=out_sb[:],
                        in0=fix_ps[:, 0:64],
                        scalar1=recip[:],
                        scalar2=None,
                        op0=MULT,
                    )
                    nc.sync.dma_start(
                        p_d[h, qi * 128:(qi + 1) * 128, :], p_out[:]
                    )
                    nc.sync.dma_start(
                        o_d[h, qi * 128:(qi + 1) * 128, :], out_sb[:]
                    )


def build_program(hloc=HLOC, qloc=QLOC):
    nqt = qloc // 128
    nc = bacc.Bacc(
        "TRN2", target_bir_lowering=False, debug=False, num_devices=NCORES
    )
    q_d = nc.dram_tensor("q", [hloc, qloc, D], f32, kind="ExternalInput").ap()
    k_d = nc.dram_tensor("k", [hloc, S, D], f32, kind="ExternalInput").ap()
    v_d = nc.dram_tensor("v", [hloc, S, D], f32, kind="ExternalInput").ap()
    m_d = nc.dram_tensor("mask", [qloc, S], i32, kind="ExternalInput").ap()
    a_d = nc.dram_tensor("alibi", [qloc, S], f32, kind="ExternalInput").ap()
    qs_d = nc.dram_tensor("qscale", [64, hloc], f32, kind="ExternalInput").ap()
    es_d = nc.dram_tensor("escale", [128, hloc], f32, kind="ExternalInput").ap()
    eb_d = nc.dram_tensor("ebias", [128, hloc], f32, kind="ExternalInput").ap()
    p_d = nc.dram_tensor("p", [hloc, qloc, S], f32, kind="ExternalOutput").ap()
    o_d = nc.dram_tensor("o", [hloc, qloc, D], f32, kind="ExternalOutput").ap()
    aps = (q_d, k_d, v_d, m_d, a_d, qs_d, es_d, eb_d, p_d, o_d)
    with tile.TileContext(nc) as tc:
        build_kernel(tc, aps, hloc=hloc, nqt=nqt)
    nc.compile()
    return nc


_CACHE = {}


def _get_program():
    if "nc" not in _CACHE:
        _CACHE["nc"] = build_program()
    return _CACHE["nc"]


def _make_in_maps(query, key, value, mask, alibi):
    slopes = _slopes()
    in_maps = []
    for c in range(NCORES):
        b, hh, qh = c >> 2, (c >> 1) & 1, c & 1
        h0, q0 = hh * HLOC, qh * QLOC
        sl = slopes[h0:h0 + HLOC]
        in_maps.append({
            "q": np.ascontiguousarray(
                query[b, h0:h0 + HLOC, q0:q0 + QLOC, :], dtype=np.float32
            ),
            "k": np.ascontiguousarray(key[b, h0:h0 + HLOC], dtype=np.float32),
            "v": np.ascontiguousarray(value[b, h0:h0 + HLOC], dtype=np.float32),
            "mask": np.ascontiguousarray(
                mask[b, 0, q0:q0 + QLOC, :], dtype=np.int32
            ),
            "alibi": np.ascontiguousarray(
                alibi[b, q0:q0 + QLOC, :], dtype=np.float32
            ),
            "qscale": np.tile(
                (0.125 / sl).astype(np.float32)[None, :], (64, 1)
            ),
            "escale": np.tile(sl.astype(np.float32)[None, :], (128, 1)),
            "ebias": np.tile(
                (-MASK_BIG * sl).astype(np.float32)[None, :], (128, 1)
            ),
        })
    return in_maps


def _gather(results):
    out = np.zeros((B, H, S, D), np.float32)
    p = np.zeros((B, H, S, S), np.float32)
    for c in range(NCORES):
        b, hh, qh = c >> 2, (c >> 1) & 1, c & 1
        h0, q0 = hh * HLOC, qh * QLOC
        out[b, h0:h0 + HLOC, q0:q0 + QLOC, :] = results[c]["o"]
        p[b, h0:h0 + HLOC, q0:q0 + QLOC, :] = results[c]["p"]
    return out, p


def kernel(query, key, value, mask, alibi):
    nc = _get_program()
    in_maps = _make_in_maps(query, key, value, mask, alibi)
    res = run_bass_kernel_spmd(nc, in_maps, core_ids=list(range(NCORES)))
    return _gather(res.results)


def _ensure_ntff_hook():
    """Wire the axon NTFF profile hook into the stub antenv package."""
    import types

    try:
        import antenv.axon_hooks  # noqa: F401

        return
    except ImportError:
        pass
    import antenv
    from trn_agent_boot.trn_boot import _ntff_profile_via_ctypes

    hook = _ntff_profile_via_ctypes("/opt/axon/libaxon_pjrt.so")
    mod = types.ModuleType("antenv.axon_hooks")
    mod.get_axon_ntff_profile_hook = lambda: hook
    mod.set_axon_ntff_profile_hook = lambda h: None
    sys.modules["antenv.axon_hooks"] = mod
    antenv.axon_hooks = mod

    import concourse.bass_utils as bu

    if not getattr(bu, "_upload_patched", False):
        orig = bu.upload_artifacts

        def _safe_upload(tmpdir):
            try:
                return orig(tmpdir)
            except Exception as e:  # no artifact bucket in this container
                return f"upload-skipped: {e}"

        bu.upload_artifacts = _safe_upload
        bu._upload_patched = True


def kernel_traced(query, key, value, mask, alibi, **kw):
    """Like kernel(), but with NTFF profiling; returns (outputs, BassKernelResults)."""
    _ensure_ntff_hook()
    nc = _get_program()
    in_maps = _make_in_maps(query, key, value, mask, alibi)
    res = run_bass_kernel_spmd(
        nc, in_maps, core_ids=list(range(NCORES)), trace=True, **kw
    )
    return _gather(res.results), res
